# revision 53
# baseline (speedup 1.0000x reference)
"""Trainium2 Bass kernel for Restormer-style transposed (channel) attention.

Per-core (1 of 8 batch elements), data-parallel over batch across 8 cores:
  qk path (feeds only the per-head gram -> softmax; fp8 quantization noise
  washes out over the 16384-pixel contraction, fp8 weight error perturbs
  logits by only ~2e-3):
    qkv_qk = W_qk @ x          (PE, fp8e4 DoubleRow: 2 K-tiles of x8/matmul)
    dw 3x3                     (PE, fp8e4 DoubleRow: 2 taps per matmul via
                                raw overlapping strided APs; 5 matmuls/tile)
    transpose (bf16) -> qki fp8 [px, 4, slot] tiles
    gram accumulated with DoubleRow over pixel pairs
  v path (full precision fp32r):
    v = W_v @ x; dw 3x3 as 9 accumulated diag matmuls over zero-padded rows;
    chunk0 (128ch) kept resident in SBUF, chunk1 (64ch) via HBM scratch
  softmax over normalized gram, batched across all 8 heads; fold the
  projection: M = W_proj @ blockdiag(A); out = M @ v (PE, fp32r)

Scheduling: slab-pipelined emission — slab s's v-dw row-tiles interleave
into slab s+1's qk-qkv phase and the v-qkv tiles interleave into the qk-dw
phase, so the PE never drains while PSUM->SBUF staging copies retire
(engines execute strictly in-order per queue).
"""
import numpy as np

NUM_HEADS = 8
C = 192
H = W = 128
HW = H * W
C3 = 3 * C            # 576
CD = C // NUM_HEADS   # 24
NCORES = 8
SLAB = 16
NSLABS = H // SLAB
EPS = 1e-12
PW = W + 4            # padded row width (2 zero cols each side)
IMG0 = 2              # image column offset within a padded row
NROWS = SLAB + 2      # stg rows per slab (1 halo row each side)
STGF = NROWS * PW     # stg free size

# qk chunks j=0..3: packed q/k channels of heads {2j, 2j+1} (96 in -> 128 slots)
# v chunks: (weight col base within v block, nch)
VCHUNKS = [(0, 128), (128, 64)]
# taps in row-major order; pairs (0,1),(2,3),(4,5),(6,7),(8,zero)
TAPS9 = [(dy, dx) for dy in (-1, 0, 1) for dx in (-1, 0, 1)]
DWT = [(0, 3), (3, 3), (6, 3), (9, 3), (12, 2), (14, 2)]


def _packed_channels(j):
    """Global qkv channel list for qk chunk j and their 32-aligned local slots."""
    chs, slots = [], []
    for b, (lo, n) in enumerate(((48 * j, 24), (192 + 48 * j, 24),
                                 (48 * j + 24, 24), (192 + 48 * j + 24, 24))):
        chs.extend(range(lo, lo + n))
        slots.extend(range(32 * b, 32 * b + n))
    return chs, slots


_CACHE = {}


def _build(reps=1):
    import concourse.bass as bass
    import concourse.mybir as mybir
    import concourse.tile as tile
    from concourse import bacc
    from contextlib import ExitStack

    dt = mybir.dt
    A = mybir.AluOpType
    AF = mybir.ActivationFunctionType
    AX = mybir.AxisListType
    DR = mybir.MatmulPerfMode.DoubleRow
    f32, bf16, f32r, f8 = dt.float32, dt.bfloat16, dt.float32r, dt.float8e4

    nc = bacc.Bacc("TRN2", num_devices=NCORES)

    xd = nc.dram_tensor("x", [C, HW], f32r, kind="ExternalInput").ap()
    wq8d = nc.dram_tensor("wq8", [128, 4 * 2 * 96], f8, kind="ExternalInput").ap()
    wqvd = nc.dram_tensor("wqv", [C, C], f32r, kind="ExternalInput").ap()
    wpTd = nc.dram_tensor("wpT", [C, C], f32, kind="ExternalInput").ap()
    dg8d = nc.dram_tensor("dg8", [96, 4 * 5 * 2 * 128], f8, kind="ExternalInput").ap()
    dgvd = nc.dram_tensor("dgv", [128, 9 * 192], f32r, kind="ExternalInput").ap()
    mskd = nc.dram_tensor("gmask", [128, 512], f32, kind="ExternalInput").ap()
    tmpd = nc.dram_tensor("tmap", [128, 4], f32, kind="ExternalInput").ap()
    eyed = nc.dram_tensor("eye", [128, 128], f32, kind="ExternalInput").ap()
    eyebd = nc.dram_tensor("eyeb", [128, 128], bf16, kind="ExternalInput").ap()
    outd = nc.dram_tensor("out", [C, HW], f32, kind="ExternalOutput").ap()

    with tile.TileContext(nc) as tc:
      with ExitStack() as _es:
        cpool = _es.enter_context(tc.tile_pool(name="const", bufs=1))
        xpool = _es.enter_context(tc.tile_pool(name="xin", bufs=2))
        x8pool = _es.enter_context(tc.tile_pool(name="x8", bufs=1))
        spool = _es.enter_context(tc.tile_pool(name="stage", bufs=1))
        spool8 = _es.enter_context(tc.tile_pool(name="stage8", bufs=2))
        bpool = _es.enter_context(tc.tile_pool(name="qkbf", bufs=3))
        qpool = _es.enter_context(tc.tile_pool(name="qki", bufs=4))
        vpool = _es.enter_context(tc.tile_pool(name="vst", bufs=1))
        vlpool = _es.enter_context(tc.tile_pool(name="vld", bufs=2))
        mpool = _es.enter_context(tc.tile_pool(name="sm", bufs=1))
        dpool = _es.enter_context(tc.tile_pool(name="dram", bufs=1, space="DRAM"))
        apool = _es.enter_context(tc.tile_pool(name="abd", bufs=1))
        opool = _es.enter_context(tc.tile_pool(name="outs", bufs=2))
        psA = _es.enter_context(tc.tile_pool(name="psA", bufs=2, space="PSUM"))
        psB = _es.enter_context(tc.tile_pool(name="psB", bufs=3, space="PSUM"))
        psG = _es.enter_context(tc.tile_pool(name="psG", bufs=1, space="PSUM"))
        psT = _es.enter_context(tc.tile_pool(name="psT", bufs=2, space="PSUM"))
        if True:
            # ---------- constants (ACT hwdge queue, ordered by first use) ----------
            wqv0 = cpool.tile([128, C], f32r, tag="wqv0")
            nc.scalar.dma_start(wqv0[:, :], wqvd[0:128, :])
            wqv1 = cpool.tile([64, C], f32r, tag="wqv1")
            nc.scalar.dma_start(wqv1[:, :], wqvd[128:192, :])
            wq8 = cpool.tile([128, 4 * 2 * 96], f8, tag="wq8")
            nc.scalar.dma_start(wq8[:, :], wq8d[:, :])
            dg8 = cpool.tile([96, 4 * 5 * 2 * 128], f8, tag="dg8")
            nc.scalar.dma_start(dg8[:, :], dg8d[:, :])
            dgv = cpool.tile([128, 9 * 192], f32r, tag="dgv")
            nc.scalar.dma_start(dgv[:, :], dgvd[:, :])
            eyeb = cpool.tile([128, 128], bf16, tag="eyeb")
            nc.scalar.dma_start(eyeb[:, :], eyebd[:, :])
            msk = cpool.tile([128, 512], f32, tag="msk")
            nc.scalar.dma_start(msk[:, :], mskd[:, :])
            tmap = cpool.tile([128, 4], f32, tag="tmap")
            nc.scalar.dma_start(tmap[:, :], tmpd[:, :])
            eye = cpool.tile([128, 128], f32, tag="eye")
            nc.scalar.dma_start(eye[:, :], eyed[:, :])
            wp0 = cpool.tile([96, C], f32, tag="wp0")
            nc.scalar.dma_start(wp0[:, :], wpTd[0:96, :])
            wp1 = cpool.tile([96, C], f32, tag="wp1")
            nc.scalar.dma_start(wp1[:, :], wpTd[96:192, :])

            # v chunk0 (128 ch) resident in SBUF; chunk1 (64 ch) via HBM
            # (a [64, HW] tile would still reserve full free bytes on all
            # 128 partitions, wasting 64 KB/partition)
            vfull0 = spool.tile([128, HW], f32r, tag="vf0")
            vtmp1 = dpool.tile([64, HW], f32r, tag="vtmp1")
            gram = psG.tile([128, 512], f32, tag="g")

            # persistent v stg buffers; pad columns zeroed once. qk stg is
            # double-buffered per slab to break the cross-slab WAR chain.
            stv = [spool.tile([nch, STGF], f32r, tag=f"stv_{j}", name=f"stv_{j}")
                   for j, (cb, nch) in enumerate(VCHUNKS)]
            def _zview(t):
                ap = t[:, :]
                if ap.dtype == f32r:
                    ap = ap.bitcast(f32)
                return ap.rearrange("p (r w) -> p r w", w=PW)

            for t in stv:
                tv = _zview(t)
                nc.gpsimd.memset(tv[:, :, 0:IMG0], 0.0)
                nc.gpsimd.memset(tv[:, :, IMG0 + W:PW], 0.0)

            wq8v = wq8[:, :].rearrange("p (j t m) -> p j t m", j=4, t=2)
            dg8v = dg8[:, :].rearrange("p (j q t m) -> p j q t m", j=4, q=5, t=2)

            def _load_slab(s):
                """Emit xs DMA loads + fp8 x8 staging for slab s."""
                row_lo = max(0, SLAB * s - 1)
                row_hi = min(H - 1, SLAB * s + SLAB)  # inclusive
                nri = row_hi - row_lo + 1
                ncols = nri * W
                col0 = row_lo * W
                xs0 = xpool.tile([128, ncols], f32r, tag="xs0")
                xs1 = xpool.tile([64, ncols], f32r, tag="xs1")
                hc = (ncols // 2) // W * W
                # split loads so the first matmuls / staging can start on the
                # first half while the second half is still in flight
                for (a, b) in ((0, hc), (hc, ncols)):
                    nc.sync.dma_start(xs0[:, a:b], xd[0:128, col0 + a:col0 + b])
                    nc.sync.dma_start(xs1[:, a:b], xd[128:192, col0 + a:col0 + b])
                # fp8 x staging for DoubleRow qkv (k-tile0 = ch 0:128,
                # k-tile1 = ch 128:192 on partitions 0:64, zeros above);
                # staged in halves so the first qk matmuls start sooner
                x8 = x8pool.tile([128, 2, ncols], f8, tag="x8")
                for (a, b) in ((0, hc), (hc, ncols)):
                    nc.gpsimd.tensor_copy(x8[:, 0, a:b], xs0[:, a:b])
                    nc.gpsimd.tensor_copy(x8[0:64, 1, a:b], xs1[:, a:b])
                    nc.gpsimd.memset(x8[64:128, 1, a:b], 0.0)
                return (xs0, xs1, x8, row_lo, ncols)

            for _rep in range(reps):
                # ---------- pass 1 ----------
                nxt = _load_slab(0)
                def _gen_dw_v(s2):
                    """v-chunk dw row-tiles as a generator (interleavable)."""
                    for jj, (cb, nch) in enumerate(VCHUNKS):
                        st_t = stv[jj]
                        vs = None
                        if jj == 1:
                            vs = vpool.tile([64, SLAB * W], f32r, tag="vs1")
                        for (lt0, nrt) in DWT:
                            L = nrt * PW - 4
                            pd = psB.tile([nch, nrt * PW], f32, tag="dw")
                            for k, (dy, dx) in enumerate(TAPS9):
                                off = (1 + lt0 + dy) * PW + IMG0 + dx
                                blk = 9 * 128 * jj + k * nch
                                nc.tensor.matmul(
                                    pd[:, IMG0:IMG0 + L],
                                    dgv[0:nch, blk:blk + nch],
                                    st_t[:, off:off + L],
                                    start=(k == 0), stop=(k == 8))
                            pdv = pd[:, :].rearrange("p (r w) -> p r w", w=PW)
                            if jj == 0:
                                c0 = (SLAB * s2 + lt0) * W
                                nc.vector.tensor_copy(
                                    vfull0[:, c0:c0 + nrt * W].rearrange(
                                        "p (r w) -> p r w", w=W),
                                    pdv[:, :, IMG0:IMG0 + W])
                            else:
                                nc.vector.tensor_copy(
                                    vs[:, lt0 * W:(lt0 + nrt) * W].rearrange(
                                        "p (r w) -> p r w", w=W),
                                    pdv[:, :, IMG0:IMG0 + W])
                            yield
                        if jj == 1:
                            nc.sync.dma_start(
                                vtmp1[:, SLAB * s2 * W:(SLAB * s2 + SLAB) * W],
                                vs[:, :])

                prev_vdw = None
                for s in range(NSLABS):
                    (xs0, xs1, x8, row_lo, ncols) = nxt
                    srow0 = row_lo - SLAB * s + 1   # stg row of first image row

                    # per-slab double-buffered qk stg (fp8)
                    st8 = [spool8.tile([96, STGF], f8, tag=f"st8_{j}",
                                       name=f"st8_{j}_{s}") for j in range(4)]
                    if True:
                        # zero the pad columns (cheap strided Pool memsets)
                        for t in st8:
                            tv = _zview(t)
                            nc.gpsimd.memset(tv[:, :, 0:IMG0], 0.0)
                            nc.gpsimd.memset(tv[:, :, IMG0 + W:PW], 0.0)
                    if s == 0 or s == NSLABS - 1:
                        zr = 0 if s == 0 else NROWS - 1
                        for t in st8:
                            tv = _zview(t)
                            nc.gpsimd.memset(tv[:, zr:zr + 1, :], 0.0)

                    tws = []
                    rem = ncols
                    while rem > 0:
                        t = min(512, rem)
                        if rem - t == 128:
                            t = 384      # keep every fp32r moving >= 256
                        tws.append(t)
                        rem -= t

                    # phase A: qk-qkv tiles interleaved with prev slab's v-dw
                    # (hides the PSUM->SBUF staging-copy drain behind PE work)
                    def _vdw_step(n=1):
                        if prev_vdw is not None:
                            for _ in range(n):
                                try:
                                    next(prev_vdw)
                                except StopIteration:
                                    break

                    ui = 0
                    for j in range(4):
                        stv8 = st8[j][:, :].rearrange("p (r w) -> p r w", w=PW)
                        t0 = 0
                        for ti, tw in enumerate(tws):
                            rr = t0 // W
                            nr4 = tw // W
                            ps = psA.tile([96, tw], f32, tag="qkv")
                            nc.tensor.matmul(ps[:, :], wq8v[:, j, :, :],
                                             x8[:, :, t0:t0 + tw],
                                             start=True, stop=True, perf_mode=DR)
                            ev = nc.vector.tensor_copy if (ti + j) % 2 == 0 \
                                else nc.scalar.copy
                            ev(stv8[:, srow0 + rr:srow0 + rr + nr4,
                                    IMG0:IMG0 + W],
                               ps[:, :].rearrange("p (r w) -> p r w", w=W))
                            t0 += tw
                            ui += 1
                            if ui % 2 == 0:
                                _vdw_step()
                    _vdw_step(16)   # drain leftovers (incl. post-yield store)
                    prev_vdw = None

                    # zero the stv out-of-image halo row at the edge slabs
                    # (must follow the previous slab's v-dw reads)
                    if s == 0 or s == NSLABS - 1:
                        zr = 0 if s == 0 else NROWS - 1
                        for t in stv:
                            tv = _zview(t)
                            nc.gpsimd.memset(tv[:, zr:zr + 1, :], 0.0)

                    # v-chunk qkv (fp32r) as a generator, interleaved into
                    # the qk-dw phase below
                    def _gen_qkv_v():
                        for jj, (cb, nch) in enumerate(VCHUNKS):
                            sv = stv[jj][:, :].rearrange("p (r w) -> p r w", w=PW)
                            t0 = 0
                            for ti, tw in enumerate(tws):
                                rr = t0 // W
                                nr4 = tw // W
                                ps = psA.tile([nch, tw], f32, tag="qkv")
                                nc.tensor.matmul(ps[:, :], wqv0[:, cb:cb + nch],
                                                 xs0[:, t0:t0 + tw],
                                                 start=True, stop=False)
                                nc.tensor.matmul(ps[:, :], wqv1[:, cb:cb + nch],
                                                 xs1[:, t0:t0 + tw],
                                                 start=False, stop=True)
                                ev = nc.vector.tensor_copy if (ti + jj) % 2 == 0 \
                                    else nc.scalar.copy
                                ev(sv[:, srow0 + rr:srow0 + rr + nr4,
                                      IMG0:IMG0 + W],
                                   ps[:, :].rearrange("p (r w) -> p r w", w=W))
                                t0 += tw
                                yield

                    vqkv = _gen_qkv_v()

                    def _vqkv_step():
                        try:
                            next(vqkv)
                        except StopIteration:
                            pass

                    # prefetch next slab's x + fp8 staging during the dw work
                    if s + 1 < NSLABS:
                        nxt = _load_slab(s + 1)

                    # ---- dw conv: qk chunks via paired-tap DoubleRow ----
                    def _flush_group(j2, gidx2, qb2):
                        ptb = psT.tile([128, 512], bf16, tag="tr")
                        for u in range(4):
                            nc.tensor.transpose(
                                ptb[:, 128 * u:128 * (u + 1)],
                                qb2[:, 128 * u:128 * (u + 1)],
                                eyeb[:, :])
                        qi = qpool.tile([128, 4, 128], f8, tag="qki")
                        ev2 = (nc.vector.tensor_copy if (gidx2 + j2) % 2 == 0
                               else nc.scalar.copy)
                        ev2(qi[:, :, :], ptb[:, :].rearrange(
                            "p (a b) -> p a b", a=4))
                        g2 = (SLAB * s + 4 * gidx2) // 2
                        for pp in range(2):
                            nc.tensor.matmul(
                                gram[:, 128 * j2:128 * (j2 + 1)],
                                qi[:, 2 * pp:2 * pp + 2, :],
                                qi[:, 2 * pp:2 * pp + 2, :],
                                start=(g2 + pp == 0),
                                stop=(g2 + pp == H // 2 - 1),
                                perf_mode=DR,
                                skip_group_check=True)

                    for j in range(4):
                        st_t = st8[j][:, :].tensor
                        qbs = {}
                        for (lt0, nrt) in DWT:
                            L = nrt * PW - 4
                            pd = psB.tile([128, nrt * PW], f32, tag="dw")
                            for p in range(5):
                                dy0, dx0 = TAPS9[2 * p]
                                off0 = (1 + lt0 + dy0) * PW + IMG0 + dx0
                                if 2 * p + 1 <= 8:
                                    dy1, dx1 = TAPS9[2 * p + 1]
                                    off1 = (1 + lt0 + dy1) * PW + IMG0 + dx1
                                else:
                                    off1 = off0 + 1   # zero-weight dummy tap
                                rhs = bass.AP(st_t, off0,
                                              [[STGF, 96], [off1 - off0, 2], [1, L]])
                                nc.tensor.matmul(pd[:, IMG0:IMG0 + L],
                                                 dg8v[:, j, p, :, :], rhs,
                                                 start=(p == 0), stop=(p == 4),
                                                 perf_mode=DR)
                            pdv = pd[:, :].rearrange("p (r w) -> p r w", w=PW)
                            # copy rows into 4-row qb groups (split at edges)
                            r = 0
                            while r < nrt:
                                gidx = (lt0 + r) // 4
                                n_in = min(nrt - r, 4 - (lt0 + r) % 4)
                                qb = qbs.get(gidx)
                                if qb is None:
                                    qb = bpool.tile([128, 512], bf16, tag="qkbf")
                                    qbs[gidx] = qb
                                qrow = (lt0 + r) % 4
                                ev3 = (nc.scalar.copy if (gidx + j) % 2 == 0
                                       else nc.vector.tensor_copy)
                                ev3(qb[:, :].rearrange("p (r w) -> p r w", w=W)[
                                        :, qrow:qrow + n_in, :],
                                    pdv[:, r:r + n_in, IMG0:IMG0 + W])
                                r += n_in
                                if qrow + n_in == 4:
                                    _flush_group(j, gidx, qb)
                            _vqkv_step()
                    for _ in range(12):
                        _vqkv_step()

                    # v-dw for this slab runs interleaved into the next slab's
                    # qkv phase (or drained after the loop for the last slab)
                    prev_vdw = _gen_dw_v(s)

                for _ in range(16):
                    try:
                        next(prev_vdw)
                    except StopIteration:
                        break
                prev_vdw = None

                # ---------- norms ----------
                gm = mpool.tile([128, 512], f32, tag="gm")
                nc.vector.tensor_tensor(gm[:, :], gram[:, :], msk[:, :], A.mult)
                s_sb = mpool.tile([128, 4], f32, tag="ssb")
                nc.vector.tensor_reduce(s_sb[:, :],
                                        gm[:, :].rearrange("p (g c) -> p g c", g=4),
                                        AX.X, A.add)
                ns = mpool.tile([128, 4], f32, tag="ns")
                nc.scalar.sqrt(ns[:, :], s_sb[:, :])
                nsc = mpool.tile([128, 4], f32, tag="nsc")
                nc.vector.tensor_scalar_max(nsc[:, :], ns[:, :], EPS)
                ry = mpool.tile([128, 4], f32, tag="ry")
                nc.vector.reciprocal(ry[:, :], nsc[:, :])
                t1 = mpool.tile([128, 4], f32, tag="t1")
                nc.vector.tensor_tensor(t1[:, :], s_sb[:, :], ry[:, :], A.mult)
                t2 = mpool.tile([128, 4], f32, tag="t2")
                nc.vector.tensor_add(t2[:, :], nsc[:, :], t1[:, :])
                ns2 = mpool.tile([128, 4], f32, tag="ns2")
                nc.vector.tensor_scalar_mul(ns2[:, :], t2[:, :], 0.5)
                ns3 = mpool.tile([128, 4], f32, tag="ns3")
                nc.vector.tensor_scalar_max(ns3[:, :], ns2[:, :], EPS)
                rn = mpool.tile([128, 4], f32, tag="rn")
                nc.vector.reciprocal(rn[:, :], ns3[:, :])
                rkt = mpool.tile([128, 4], f32, tag="rkt")
                nc.vector.tensor_tensor(rkt[:, :], rn[:, :], tmap[:, :], A.mult)
                rq = mpool.tile([24, 8], f32, tag="rq")
                nc.sync.dma_start(rq[0:24, 0:7:2], rn[0:24, 0:4])
                nc.sync.dma_start(rq[0:24, 1:8:2], rn[64:88, 0:4])

                # ---------- softmax + A blockdiag (batched over heads) ------
                a0 = apool.tile([96, C], f32, tag="a0")
                a1 = apool.tile([96, C], f32, tag="a1")
                nc.vector.memset(a0[:, :], 0.0)
                nc.vector.memset(a1[:, :], 0.0)
                bt = mpool.tile([128, 8 * CD], f32, tag="bt")
                sp = mpool.tile([CD, 8 * CD], f32, tag="sp")
                for h in range(NUM_HEADS):
                    p = h // 2
                    if h % 2 == 0:
                        kbase, qcol = 32, 0
                    else:
                        kbase, qcol = 96, 64
                    nc.vector.tensor_scalar_mul(
                        bt[kbase:kbase + CD, CD * h:CD * (h + 1)],
                        gram[kbase:kbase + CD, 128 * p + qcol:128 * p + qcol + CD],
                        rkt[kbase:kbase + CD, p:p + 1])
                    ptr = psA.tile([CD, CD], f32, tag="qkv")
                    nc.tensor.transpose(ptr[:, :],
                                        bt[kbase:kbase + CD, CD * h:CD * (h + 1)],
                                        eye[kbase:kbase + CD, kbase:kbase + CD],
                                        tile_position=(kbase, 0))
                    ev = nc.vector.tensor_copy if h % 2 == 0 else nc.scalar.copy
                    ev(sp[:, CD * h:CD * (h + 1)], ptr[:, :])
                sp3 = sp[:, :].rearrange("p (h c) -> p h c", h=8)
                rqb = rq[0:CD, 0:8].unsqueeze(2).broadcast_to((CD, 8, CD))
                ls = mpool.tile([CD, 8 * CD], f32, tag="ls")
                ls3 = ls[:, :].rearrange("p (h c) -> p h c", h=8)
                nc.vector.tensor_tensor(ls3, sp3, rqb, A.mult)
                mx = mpool.tile([CD, 8], f32, tag="mx")
                nc.vector.tensor_reduce(mx[:, :], ls3, AX.X, A.max)
                eb = mpool.tile([CD, 8 * CD], f32, tag="eb")
                eb3 = eb[:, :].rearrange("p (h c) -> p h c", h=8)
                mxb = mx[0:CD, 0:8].unsqueeze(2).broadcast_to((CD, 8, CD))
                nc.vector.tensor_tensor(eb3, ls3, mxb, A.subtract)
                es = mpool.tile([CD, 8 * CD], f32, tag="es")
                nc.scalar.activation(es[:, :], eb[:, :], AF.Exp,
                                     bias=0.0, scale=1.0)
                se = mpool.tile([CD, 8], f32, tag="se")
                nc.vector.tensor_reduce(se[:, :],
                                        es[:, :].rearrange("p (h c) -> p h c", h=8),
                                        AX.X, A.add)
                rse = mpool.tile([CD, 8], f32, tag="rse")
                nc.vector.reciprocal(rse[:, :], se[:, :])
                ahc = mpool.tile([CD, 8 * CD], f32, tag="ahc")
                ahc3 = ahc[:, :].rearrange("p (h c) -> p h c", h=8)
                rseb = rse[0:CD, 0:8].unsqueeze(2).broadcast_to((CD, 8, CD))
                nc.vector.tensor_tensor(ahc3, es[:, :].rearrange(
                    "p (h c) -> p h c", h=8), rseb, A.mult)
                # scatter the 24x24 blocks into the blockdiag layout
                for h in range(NUM_HEADS):
                    adst = a0 if h < 4 else a1
                    r0 = CD * (h % 4)
                    nc.sync.dma_start(adst[r0:r0 + CD, CD * h:CD * (h + 1)],
                                      ahc[:, CD * h:CD * (h + 1)])

                # ---------- M^T = A_bd^T @ W_proj^T ----------
                mt0 = cpool.tile([128, C], f32r, tag="mt0")
                mt1 = cpool.tile([64, C], f32r, tag="mt1")
                pmt0 = psA.tile([128, C], f32, tag="qkv")
                nc.tensor.matmul(pmt0[:, :], a0[:, 0:128], wp0[:, :],
                                 start=True, stop=False)
                nc.tensor.matmul(pmt0[:, :], a1[:, 0:128], wp1[:, :],
                                 start=False, stop=True)
                nc.scalar.copy(mt0[:, :], pmt0[:, :])
                pmt1 = psA.tile([64, C], f32, tag="qkv")
                nc.tensor.matmul(pmt1[:, :], a0[:, 128:192], wp0[:, :],
                                 start=True, stop=False)
                nc.tensor.matmul(pmt1[:, :], a1[:, 128:192], wp1[:, :],
                                 start=False, stop=True)
                nc.scalar.copy(mt1[:, :], pmt1[:, :])

                # ---------- pass 2: out = M @ v (v0 in SBUF, v1 from HBM) ----
                BK = 1024
                vls = {}

                def _load_vl(b0):
                    t = vlpool.tile([64, BK], f32r, tag="vl1")
                    nc.sync.dma_start(t[:, :], vtmp1[:, b0:b0 + BK])
                    vls[b0] = t

                _load_vl(0)
                _load_vl(BK)
                for b0 in range(0, HW, BK):
                    if b0 + 2 * BK < HW:
                        _load_vl(b0 + 2 * BK)
                    vl1 = vls.pop(b0)
                    oa = opool.tile([128, BK], f32, tag="oa")
                    ob = opool.tile([64, BK], f32, tag="ob")
                    for t0 in range(b0, b0 + BK, 512):
                        u0 = t0 - b0
                        pa = psA.tile([128, 512], f32, tag="qkv")
                        nc.tensor.matmul(pa[:, :], mt0[:, 0:128],
                                         vfull0[:, t0:t0 + 512],
                                         start=True, stop=False)
                        nc.tensor.matmul(pa[:, :], mt1[:, 0:128],
                                         vl1[:, u0:u0 + 512],
                                         start=False, stop=True)
                        nc.scalar.copy(oa[:, u0:u0 + 512], pa[:, :])
                        pb = psB.tile([64, 512], f32, tag="dw")
                        nc.tensor.matmul(pb[:, :], mt0[:, 128:192],
                                         vfull0[:, t0:t0 + 512],
                                         start=True, stop=False)
                        nc.tensor.matmul(pb[:, :], mt1[:, 128:192],
                                         vl1[:, u0:u0 + 512],
                                         start=False, stop=True)
                        nc.scalar.copy(ob[:, u0:u0 + 512], pb[:, :])
                    nc.scalar.dma_start(outd[0:128, b0:b0 + BK], oa[:, :])
                    nc.scalar.dma_start(outd[128:192, b0:b0 + BK], ob[:, :])

    nc.compile()
    return nc


def _host_consts(w_qkv, w_dw, w_proj, temperature):
    import ml_dtypes
    fp8 = ml_dtypes.float8_e4m3
    wqT_plain = np.ascontiguousarray(np.asarray(w_qkv, np.float32).T)   # [192, 576]
    wpT = np.ascontiguousarray(np.asarray(w_proj, np.float32).T)        # [192, 192]
    wd = np.asarray(w_dw, dtype=np.float32).reshape(C3, 3, 3)

    # fp8 DoubleRow qkv weights for qk chunks: [128, j, kt, m]
    wq8 = np.zeros((128, 4, 2, 96), dtype=np.float32)
    # fp8 paired-tap dw weights: [96, j, pair, kt, slot]
    dg8 = np.zeros((96, 4, 5, 2, 128), dtype=np.float32)
    for j in range(4):
        chs, slots = _packed_channels(j)
        chs = np.asarray(chs)
        slots = np.asarray(slots)
        wq8[0:128, j, 0, :] = wqT_plain[0:128][:, chs]
        wq8[0:64, j, 1, :] = wqT_plain[128:192][:, chs]
        for p in range(5):
            for kt in range(2):
                t = 2 * p + kt
                if t > 8:
                    continue
                dy, dx = TAPS9[t]
                dg8[np.arange(96), j, p, kt, slots] = wd[chs, dy + 1, dx + 1]

    # v chunk weights
    wqv = np.ascontiguousarray(wqT_plain[:, 384:576])                   # [192, 192]
    dgv = np.zeros((128, 9 * 192), dtype=np.float32)
    for jj, (cb, nch) in enumerate(VCHUNKS):
        for k, (dy, dx) in enumerate(TAPS9):
            col = 9 * 128 * jj + k * nch
            idx = np.arange(nch)
            dgv[idx, col + idx] = wd[384 + cb + idx, dy + 1, dx + 1]

    gmask = np.tile(np.eye(128, dtype=np.float32), (1, 4))              # [128, 512]
    tmapv = np.ones((128, 4), dtype=np.float32)
    tf = np.asarray(temperature, dtype=np.float32).reshape(-1)
    for p in range(4):
        tmapv[32:56, p] = tf[2 * p]
        tmapv[96:120, p] = tf[2 * p + 1]
    eyev = np.eye(128, dtype=np.float32)
    eyebv = np.eye(128).astype(ml_dtypes.bfloat16)
    return dict(wq8=wq8.reshape(128, -1).astype(fp8),
                dg8=dg8.reshape(96, -1).astype(fp8),
                wqv=wqv, dgv=dgv, wpT=wpT, gmask=gmask, tmap=tmapv,
                eye=eyev, eyeb=eyebv)


def kernel(x, w_qkv, w_dw, w_proj, temperature, _trace=False):
    from concourse.bass_utils import run_bass_kernel_spmd

    if "nc" not in _CACHE:
        _CACHE["nc"] = _build()
    nc = _CACHE["nc"]

    consts = _host_consts(w_qkv, w_dw, w_proj, temperature)
    xr = np.ascontiguousarray(np.asarray(x, dtype=np.float32).reshape(NCORES, C, HW))
    in_maps = []
    for b in range(NCORES):
        m = {"x": xr[b]}
        m.update(consts)
        in_maps.append(m)

    try:
        br = run_bass_kernel_spmd(nc, in_maps, core_ids=list(range(NCORES)),
                                  trace=_trace)
    except ModuleNotFoundError:
        br = run_bass_kernel_spmd(nc, in_maps, core_ids=list(range(NCORES)),
                                  trace=False)
    out = np.stack([r["out"] for r in br.results], axis=0).reshape(NCORES, C, H, W)
    _CACHE["last_results"] = br
    return out


# revision 54
# speedup vs baseline: 1.0198x; 1.0198x over previous
"""Trainium2 Bass kernel for Restormer-style transposed (channel) attention.

Per-core (1 of 8 batch elements), data-parallel over batch across 8 cores:
  qk path (feeds only the per-head gram -> softmax; fp8 quantization noise
  washes out over the 16384-pixel contraction, fp8 weight error perturbs
  logits by only ~2e-3):
    qkv_qk = W_qk @ x          (PE, fp8e4 DoubleRow: 2 K-tiles of x8/matmul)
    dw 3x3                     (PE, fp8e4 DoubleRow: 2 taps per matmul via
                                raw overlapping strided APs; 5 matmuls/tile)
    transpose (bf16) -> qki fp8 [px, 4, slot] tiles
    gram accumulated with DoubleRow over pixel pairs
  v path (full precision fp32r):
    v = W_v @ x; dw 3x3 as 9 accumulated diag matmuls over zero-padded rows;
    chunk0 (128ch) kept resident in SBUF, chunk1 (64ch) via HBM scratch
  softmax over normalized gram, batched across all 8 heads; fold the
  projection: M = W_proj @ blockdiag(A); out = M @ v (PE, fp32r)

Scheduling: slab-pipelined emission — slab s's v-dw row-tiles interleave
into slab s+1's qk-qkv phase and the v-qkv tiles interleave into the qk-dw
phase, so the PE never drains while PSUM->SBUF staging copies retire
(engines execute strictly in-order per queue).
"""
import numpy as np

NUM_HEADS = 8
C = 192
H = W = 128
HW = H * W
C3 = 3 * C            # 576
CD = C // NUM_HEADS   # 24
NCORES = 8
SLAB = 16
NSLABS = H // SLAB
EPS = 1e-12
PW = W + 4            # padded row width (2 zero cols each side)
IMG0 = 2              # image column offset within a padded row
NROWS = SLAB + 2      # stg rows per slab (1 halo row each side)
STGF = NROWS * PW     # stg free size

# qk chunks j=0..3: packed q/k channels of heads {2j, 2j+1} (96 in -> 128 slots)
# v chunks: (weight col base within v block, nch)
VCHUNKS = [(0, 128), (128, 64)]
# taps in row-major order; pairs (0,1),(2,3),(4,5),(6,7),(8,zero)
TAPS9 = [(dy, dx) for dy in (-1, 0, 1) for dx in (-1, 0, 1)]
DWT = [(0, 3), (3, 3), (6, 3), (9, 3), (12, 2), (14, 2)]


def _packed_channels(j):
    """Global qkv channel list for qk chunk j and their 32-aligned local slots."""
    chs, slots = [], []
    for b, (lo, n) in enumerate(((48 * j, 24), (192 + 48 * j, 24),
                                 (48 * j + 24, 24), (192 + 48 * j + 24, 24))):
        chs.extend(range(lo, lo + n))
        slots.extend(range(32 * b, 32 * b + n))
    return chs, slots


_CACHE = {}


def _build(reps=1):
    import concourse.bass as bass
    import concourse.mybir as mybir
    import concourse.tile as tile
    from concourse import bacc
    from contextlib import ExitStack

    dt = mybir.dt
    A = mybir.AluOpType
    AF = mybir.ActivationFunctionType
    AX = mybir.AxisListType
    DR = mybir.MatmulPerfMode.DoubleRow
    f32, bf16, f32r, f8 = dt.float32, dt.bfloat16, dt.float32r, dt.float8e4

    nc = bacc.Bacc("TRN2", num_devices=NCORES)

    xd = nc.dram_tensor("x", [C, HW], f32r, kind="ExternalInput").ap()
    wq8d = nc.dram_tensor("wq8", [128, 4 * 2 * 96], f8, kind="ExternalInput").ap()
    wqvd = nc.dram_tensor("wqv", [C, C], f32r, kind="ExternalInput").ap()
    wpTd = nc.dram_tensor("wpT", [C, C], f32, kind="ExternalInput").ap()
    dg8d = nc.dram_tensor("dg8", [96, 4 * 5 * 2 * 128], f8, kind="ExternalInput").ap()
    dgvd = nc.dram_tensor("dgv", [128, 9 * 192], f32r, kind="ExternalInput").ap()
    mskd = nc.dram_tensor("gmask", [128, 512], f32, kind="ExternalInput").ap()
    tmpd = nc.dram_tensor("tmap", [128, 4], f32, kind="ExternalInput").ap()
    eyed = nc.dram_tensor("eye", [128, 128], f32, kind="ExternalInput").ap()
    eyebd = nc.dram_tensor("eyeb", [128, 128], bf16, kind="ExternalInput").ap()
    outd = nc.dram_tensor("out", [C, HW], f32, kind="ExternalOutput").ap()

    with tile.TileContext(nc) as tc:
      with ExitStack() as _es:
        cpool = _es.enter_context(tc.tile_pool(name="const", bufs=1))
        xpool = _es.enter_context(tc.tile_pool(name="xin", bufs=2))
        x8pool = _es.enter_context(tc.tile_pool(name="x8", bufs=1))
        spool = _es.enter_context(tc.tile_pool(name="stage", bufs=1))
        spool8 = _es.enter_context(tc.tile_pool(name="stage8", bufs=2))
        bpool = _es.enter_context(tc.tile_pool(name="qkbf", bufs=3))
        qpool = _es.enter_context(tc.tile_pool(name="qki", bufs=4))
        vpool = _es.enter_context(tc.tile_pool(name="vst", bufs=1))
        vlpool = _es.enter_context(tc.tile_pool(name="vld", bufs=2))
        mpool = _es.enter_context(tc.tile_pool(name="sm", bufs=1))
        dpool = _es.enter_context(tc.tile_pool(name="dram", bufs=1, space="DRAM"))
        apool = _es.enter_context(tc.tile_pool(name="abd", bufs=1))
        opool = _es.enter_context(tc.tile_pool(name="outs", bufs=2))
        psA = _es.enter_context(tc.tile_pool(name="psA", bufs=2, space="PSUM"))
        psB = _es.enter_context(tc.tile_pool(name="psB", bufs=3, space="PSUM"))
        psG = _es.enter_context(tc.tile_pool(name="psG", bufs=1, space="PSUM"))
        psT = _es.enter_context(tc.tile_pool(name="psT", bufs=2, space="PSUM"))
        if True:
            # ---------- constants (ACT hwdge queue, ordered by first use) ----------
            wqv0 = cpool.tile([128, C], f32r, tag="wqv0")
            nc.scalar.dma_start(wqv0[:, :], wqvd[0:128, :])
            wqv1 = cpool.tile([64, C], f32r, tag="wqv1")
            nc.scalar.dma_start(wqv1[:, :], wqvd[128:192, :])
            wq8 = cpool.tile([128, 4 * 2 * 96], f8, tag="wq8")
            nc.scalar.dma_start(wq8[:, :], wq8d[:, :])
            dg8 = cpool.tile([96, 4 * 5 * 2 * 128], f8, tag="dg8")
            nc.scalar.dma_start(dg8[:, :], dg8d[:, :])
            dgv = cpool.tile([128, 9 * 192], f32r, tag="dgv")
            nc.scalar.dma_start(dgv[:, :], dgvd[:, :])
            eyeb = cpool.tile([128, 128], bf16, tag="eyeb")
            nc.scalar.dma_start(eyeb[:, :], eyebd[:, :])
            msk = cpool.tile([128, 512], f32, tag="msk")
            nc.scalar.dma_start(msk[:, :], mskd[:, :])
            tmap = cpool.tile([128, 4], f32, tag="tmap")
            nc.scalar.dma_start(tmap[:, :], tmpd[:, :])
            eye = cpool.tile([128, 128], f32, tag="eye")
            nc.scalar.dma_start(eye[:, :], eyed[:, :])
            wp0 = cpool.tile([96, C], f32, tag="wp0")
            nc.scalar.dma_start(wp0[:, :], wpTd[0:96, :])
            wp1 = cpool.tile([96, C], f32, tag="wp1")
            nc.scalar.dma_start(wp1[:, :], wpTd[96:192, :])

            # v chunk0 (128 ch) resident in SBUF; chunk1 (64 ch) via HBM
            # (a [64, HW] tile would still reserve full free bytes on all
            # 128 partitions, wasting 64 KB/partition)
            vfull0 = spool.tile([128, HW], f32r, tag="vf0")
            vtmp1 = dpool.tile([64, HW], f32r, tag="vtmp1")
            gram = psG.tile([128, 512], f32, tag="g")

            # persistent v stg buffers; pad columns zeroed once. qk stg is
            # double-buffered per slab to break the cross-slab WAR chain.
            stv = [spool.tile([nch, STGF], f32r, tag=f"stv_{j}", name=f"stv_{j}")
                   for j, (cb, nch) in enumerate(VCHUNKS)]
            def _zview(t):
                ap = t[:, :]
                if ap.dtype == f32r:
                    ap = ap.bitcast(f32)
                return ap.rearrange("p (r w) -> p r w", w=PW)

            for t in stv:
                tv = _zview(t)
                nc.gpsimd.memset(tv[:, :, 0:IMG0], 0.0)
                nc.gpsimd.memset(tv[:, :, IMG0 + W:PW], 0.0)

            wq8v = wq8[:, :].rearrange("p (j t m) -> p j t m", j=4, t=2)
            dg8v = dg8[:, :].rearrange("p (j q t m) -> p j q t m", j=4, q=5, t=2)

            def _load_slab(s):
                """Emit xs DMA loads + fp8 x8 staging for slab s."""
                row_lo = max(0, SLAB * s - 1)
                row_hi = min(H - 1, SLAB * s + SLAB)  # inclusive
                nri = row_hi - row_lo + 1
                ncols = nri * W
                col0 = row_lo * W
                xs0 = xpool.tile([128, ncols], f32r, tag="xs0")
                xs1 = xpool.tile([64, ncols], f32r, tag="xs1")
                hc = (ncols // 2) // W * W
                # split loads so the first matmuls / staging can start on the
                # first half while the second half is still in flight
                for (a, b) in ((0, hc), (hc, ncols)):
                    nc.sync.dma_start(xs0[:, a:b], xd[0:128, col0 + a:col0 + b])
                    nc.sync.dma_start(xs1[:, a:b], xd[128:192, col0 + a:col0 + b])
                # fp8 x staging for DoubleRow qkv (k-tile0 = ch 0:128,
                # k-tile1 = ch 128:192 on partitions 0:64, zeros above);
                # staged in halves so the first qk matmuls start sooner
                x8 = x8pool.tile([128, 2, ncols], f8, tag="x8")
                for (a, b) in ((0, hc), (hc, ncols)):
                    nc.gpsimd.tensor_copy(x8[:, 0, a:b], xs0[:, a:b])
                    nc.gpsimd.tensor_copy(x8[0:64, 1, a:b], xs1[:, a:b])
                    nc.gpsimd.memset(x8[64:128, 1, a:b], 0.0)
                return (xs0, xs1, x8, row_lo, ncols)

            for _rep in range(reps):
                # ---------- pass 1 ----------
                nxt = _load_slab(0)
                def _gen_dw_v(s2):
                    """v-chunk dw row-tiles as a generator (interleavable)."""
                    for jj, (cb, nch) in enumerate(VCHUNKS):
                        st_t = stv[jj]
                        vs = None
                        if jj == 1:
                            vs = vpool.tile([64, SLAB * W], f32r, tag="vs1")
                        for (lt0, nrt) in DWT:
                            L = nrt * PW - 4
                            pd = psB.tile([nch, nrt * PW], f32, tag="dw")
                            for k, (dy, dx) in enumerate(TAPS9):
                                off = (1 + lt0 + dy) * PW + IMG0 + dx
                                blk = 9 * 128 * jj + k * nch
                                nc.tensor.matmul(
                                    pd[:, IMG0:IMG0 + L],
                                    dgv[0:nch, blk:blk + nch],
                                    st_t[:, off:off + L],
                                    start=(k == 0), stop=(k == 8))
                            pdv = pd[:, :].rearrange("p (r w) -> p r w", w=PW)
                            if jj == 0:
                                c0 = (SLAB * s2 + lt0) * W
                                nc.vector.tensor_copy(
                                    vfull0[:, c0:c0 + nrt * W].rearrange(
                                        "p (r w) -> p r w", w=W),
                                    pdv[:, :, IMG0:IMG0 + W])
                            else:
                                nc.vector.tensor_copy(
                                    vs[:, lt0 * W:(lt0 + nrt) * W].rearrange(
                                        "p (r w) -> p r w", w=W),
                                    pdv[:, :, IMG0:IMG0 + W])
                            yield
                        if jj == 1:
                            nc.sync.dma_start(
                                vtmp1[:, SLAB * s2 * W:(SLAB * s2 + SLAB) * W],
                                vs[:, :])

                prev_vdw = None
                for s in range(NSLABS):
                    (xs0, xs1, x8, row_lo, ncols) = nxt
                    srow0 = row_lo - SLAB * s + 1   # stg row of first image row

                    # per-slab double-buffered qk stg (fp8)
                    st8 = [spool8.tile([96, STGF], f8, tag=f"st8_{j}",
                                       name=f"st8_{j}_{s}") for j in range(4)]
                    if True:
                        # zero the pad columns (cheap strided Pool memsets)
                        for t in st8:
                            tv = _zview(t)
                            nc.gpsimd.memset(tv[:, :, 0:IMG0], 0.0)
                            nc.gpsimd.memset(tv[:, :, IMG0 + W:PW], 0.0)
                    if s == 0 or s == NSLABS - 1:
                        zr = 0 if s == 0 else NROWS - 1
                        for t in st8:
                            tv = _zview(t)
                            nc.gpsimd.memset(tv[:, zr:zr + 1, :], 0.0)

                    tws = []
                    rem = ncols
                    while rem > 0:
                        t = min(512, rem)
                        if rem - t == 128:
                            t = 384      # keep every fp32r moving >= 256
                        tws.append(t)
                        rem -= t

                    # phase A: qk-qkv tiles interleaved with prev slab's v-dw
                    # (hides the PSUM->SBUF staging-copy drain behind PE work)
                    def _vdw_step(n=1):
                        if prev_vdw is not None:
                            for _ in range(n):
                                try:
                                    next(prev_vdw)
                                except StopIteration:
                                    break

                    ui = 0
                    for j in range(4):
                        stv8 = st8[j][:, :].rearrange("p (r w) -> p r w", w=PW)
                        t0 = 0
                        for ti, tw in enumerate(tws):
                            rr = t0 // W
                            nr4 = tw // W
                            ps = psA.tile([96, tw], f32, tag="qkv")
                            nc.tensor.matmul(ps[:, :], wq8v[:, j, :, :],
                                             x8[:, :, t0:t0 + tw],
                                             start=True, stop=True, perf_mode=DR)
                            ev = nc.vector.tensor_copy if (ti + j) % 2 == 0 \
                                else nc.scalar.copy
                            ev(stv8[:, srow0 + rr:srow0 + rr + nr4,
                                    IMG0:IMG0 + W],
                               ps[:, :].rearrange("p (r w) -> p r w", w=W))
                            t0 += tw
                            ui += 1
                            if ui % 2 == 0:
                                _vdw_step()
                    _vdw_step(16)   # drain leftovers (incl. post-yield store)
                    prev_vdw = None

                    # zero the stv out-of-image halo row at the edge slabs
                    # (must follow the previous slab's v-dw reads)
                    if s == 0 or s == NSLABS - 1:
                        zr = 0 if s == 0 else NROWS - 1
                        for t in stv:
                            tv = _zview(t)
                            nc.gpsimd.memset(tv[:, zr:zr + 1, :], 0.0)

                    # v-chunk qkv (fp32r) as a generator, interleaved into
                    # the qk-dw phase below
                    def _gen_qkv_v():
                        for jj, (cb, nch) in enumerate(VCHUNKS):
                            sv = stv[jj][:, :].rearrange("p (r w) -> p r w", w=PW)
                            t0 = 0
                            for ti, tw in enumerate(tws):
                                rr = t0 // W
                                nr4 = tw // W
                                ps = psA.tile([nch, tw], f32, tag="qkv")
                                nc.tensor.matmul(ps[:, :], wqv0[:, cb:cb + nch],
                                                 xs0[:, t0:t0 + tw],
                                                 start=True, stop=False)
                                nc.tensor.matmul(ps[:, :], wqv1[:, cb:cb + nch],
                                                 xs1[:, t0:t0 + tw],
                                                 start=False, stop=True)
                                ev = nc.vector.tensor_copy if (ti + jj) % 2 == 0 \
                                    else nc.scalar.copy
                                ev(sv[:, srow0 + rr:srow0 + rr + nr4,
                                      IMG0:IMG0 + W],
                                   ps[:, :].rearrange("p (r w) -> p r w", w=W))
                                t0 += tw
                                yield

                    vqkv = _gen_qkv_v()

                    def _vqkv_step():
                        try:
                            next(vqkv)
                        except StopIteration:
                            pass

                    # prefetch next slab's x + fp8 staging during the dw work
                    if s + 1 < NSLABS:
                        nxt = _load_slab(s + 1)

                    # ---- dw conv: qk chunks via paired-tap DoubleRow ----
                    def _flush_group(j2, gidx2, qb2):
                        ptb = psT.tile([128, 512], bf16, tag="tr")
                        for u in range(4):
                            nc.tensor.transpose(
                                ptb[:, 128 * u:128 * (u + 1)],
                                qb2[:, 128 * u:128 * (u + 1)],
                                eyeb[:, :])
                        qi = qpool.tile([128, 4, 128], f8, tag="qki")
                        ev2 = (nc.vector.tensor_copy if (gidx2 + j2) % 2 == 0
                               else nc.scalar.copy)
                        ev2(qi[:, :, :], ptb[:, :].rearrange(
                            "p (a b) -> p a b", a=4))
                        g2 = (SLAB * s + 4 * gidx2) // 2
                        for pp in range(2):
                            nc.tensor.matmul(
                                gram[:, 128 * j2:128 * (j2 + 1)],
                                qi[:, 2 * pp:2 * pp + 2, :],
                                qi[:, 2 * pp:2 * pp + 2, :],
                                start=(g2 + pp == 0),
                                stop=(g2 + pp == H // 2 - 1),
                                perf_mode=DR,
                                skip_group_check=True)

                    for j in range(4):
                        st_t = st8[j][:, :].tensor
                        qbs = {}
                        for (lt0, nrt) in DWT:
                            L = nrt * PW - 4
                            pd = psB.tile([128, nrt * PW], f32, tag="dw")
                            for p in range(5):
                                dy0, dx0 = TAPS9[2 * p]
                                off0 = (1 + lt0 + dy0) * PW + IMG0 + dx0
                                if 2 * p + 1 <= 8:
                                    dy1, dx1 = TAPS9[2 * p + 1]
                                    off1 = (1 + lt0 + dy1) * PW + IMG0 + dx1
                                else:
                                    off1 = off0 + 1   # zero-weight dummy tap
                                rhs = bass.AP(st_t, off0,
                                              [[STGF, 96], [off1 - off0, 2], [1, L]])
                                nc.tensor.matmul(pd[:, IMG0:IMG0 + L],
                                                 dg8v[:, j, p, :, :], rhs,
                                                 start=(p == 0), stop=(p == 4),
                                                 perf_mode=DR)
                            pdv = pd[:, :].rearrange("p (r w) -> p r w", w=PW)
                            # copy rows into 4-row qb groups (split at edges)
                            r = 0
                            while r < nrt:
                                gidx = (lt0 + r) // 4
                                n_in = min(nrt - r, 4 - (lt0 + r) % 4)
                                qb = qbs.get(gidx)
                                if qb is None:
                                    qb = bpool.tile([128, 512], bf16, tag="qkbf")
                                    qbs[gidx] = qb
                                qrow = (lt0 + r) % 4
                                ev3 = (nc.scalar.copy if (gidx + j) % 2 == 0
                                       else nc.vector.tensor_copy)
                                ev3(qb[:, :].rearrange("p (r w) -> p r w", w=W)[
                                        :, qrow:qrow + n_in, :],
                                    pdv[:, r:r + n_in, IMG0:IMG0 + W])
                                r += n_in
                                if qrow + n_in == 4:
                                    _flush_group(j, gidx, qb)
                            _vqkv_step()
                    for _ in range(12):
                        _vqkv_step()

                    # v-dw for this slab runs interleaved into the next slab's
                    # qkv phase (or drained after the loop for the last slab)
                    prev_vdw = _gen_dw_v(s)

                for _ in range(16):
                    try:
                        next(prev_vdw)
                    except StopIteration:
                        break
                prev_vdw = None

                # ---------- norms ----------
                gm = mpool.tile([128, 512], f32, tag="gm")
                nc.vector.tensor_tensor(gm[:, :], gram[:, :], msk[:, :], A.mult)
                s_sb = mpool.tile([128, 4], f32, tag="ssb")
                nc.vector.tensor_reduce(s_sb[:, :],
                                        gm[:, :].rearrange("p (g c) -> p g c", g=4),
                                        AX.X, A.add)
                ns = mpool.tile([128, 4], f32, tag="ns")
                nc.scalar.sqrt(ns[:, :], s_sb[:, :])
                nsc = mpool.tile([128, 4], f32, tag="nsc")
                nc.vector.tensor_scalar_max(nsc[:, :], ns[:, :], EPS)
                ry = mpool.tile([128, 4], f32, tag="ry")
                nc.vector.reciprocal(ry[:, :], nsc[:, :])
                t1 = mpool.tile([128, 4], f32, tag="t1")
                nc.vector.tensor_tensor(t1[:, :], s_sb[:, :], ry[:, :], A.mult)
                t2 = mpool.tile([128, 4], f32, tag="t2")
                nc.vector.tensor_add(t2[:, :], nsc[:, :], t1[:, :])
                ns2 = mpool.tile([128, 4], f32, tag="ns2")
                nc.vector.tensor_scalar_mul(ns2[:, :], t2[:, :], 0.5)
                ns3 = mpool.tile([128, 4], f32, tag="ns3")
                nc.vector.tensor_scalar_max(ns3[:, :], ns2[:, :], EPS)
                rn = mpool.tile([128, 4], f32, tag="rn")
                nc.vector.reciprocal(rn[:, :], ns3[:, :])
                rkt = mpool.tile([128, 4], f32, tag="rkt")
                nc.vector.tensor_tensor(rkt[:, :], rn[:, :], tmap[:, :], A.mult)
                rq = mpool.tile([24, 8], f32, tag="rq")
                nc.sync.dma_start(rq[0:24, 0:7:2], rn[0:24, 0:4])
                nc.sync.dma_start(rq[0:24, 1:8:2], rn[64:88, 0:4])

                # ---------- softmax + A blockdiag (batched over heads) ------
                a0 = apool.tile([96, C], f32, tag="a0")
                a1 = apool.tile([96, C], f32, tag="a1")
                nc.vector.memset(a0[:, :], 0.0)
                nc.vector.memset(a1[:, :], 0.0)
                bt = mpool.tile([128, 8 * CD], f32, tag="bt")
                sp = mpool.tile([CD, 8 * CD], f32, tag="sp")
                for h in range(NUM_HEADS):
                    p = h // 2
                    if h % 2 == 0:
                        kbase, qcol = 32, 0
                    else:
                        kbase, qcol = 96, 64
                    nc.vector.tensor_scalar_mul(
                        bt[kbase:kbase + CD, CD * h:CD * (h + 1)],
                        gram[kbase:kbase + CD, 128 * p + qcol:128 * p + qcol + CD],
                        rkt[kbase:kbase + CD, p:p + 1])
                    ptr = psA.tile([CD, CD], f32, tag="qkv")
                    nc.tensor.transpose(ptr[:, :],
                                        bt[kbase:kbase + CD, CD * h:CD * (h + 1)],
                                        eye[kbase:kbase + CD, kbase:kbase + CD],
                                        tile_position=(kbase, 0))
                    ev = nc.vector.tensor_copy if h % 2 == 0 else nc.scalar.copy
                    ev(sp[:, CD * h:CD * (h + 1)], ptr[:, :])
                sp3 = sp[:, :].rearrange("p (h c) -> p h c", h=8)
                rqb = rq[0:CD, 0:8].unsqueeze(2).broadcast_to((CD, 8, CD))
                ls = mpool.tile([CD, 8 * CD], f32, tag="ls")
                ls3 = ls[:, :].rearrange("p (h c) -> p h c", h=8)
                nc.vector.tensor_tensor(ls3, sp3, rqb, A.mult)
                mx = mpool.tile([CD, 8], f32, tag="mx")
                nc.vector.tensor_reduce(mx[:, :], ls3, AX.X, A.max)
                eb = mpool.tile([CD, 8 * CD], f32, tag="eb")
                eb3 = eb[:, :].rearrange("p (h c) -> p h c", h=8)
                mxb = mx[0:CD, 0:8].unsqueeze(2).broadcast_to((CD, 8, CD))
                nc.vector.tensor_tensor(eb3, ls3, mxb, A.subtract)
                es = mpool.tile([CD, 8 * CD], f32, tag="es")
                nc.scalar.activation(es[:, :], eb[:, :], AF.Exp,
                                     bias=0.0, scale=1.0)
                se = mpool.tile([CD, 8], f32, tag="se")
                nc.vector.tensor_reduce(se[:, :],
                                        es[:, :].rearrange("p (h c) -> p h c", h=8),
                                        AX.X, A.add)
                rse = mpool.tile([CD, 8], f32, tag="rse")
                nc.vector.reciprocal(rse[:, :], se[:, :])
                ahc = mpool.tile([CD, 8 * CD], f32, tag="ahc")
                ahc3 = ahc[:, :].rearrange("p (h c) -> p h c", h=8)
                rseb = rse[0:CD, 0:8].unsqueeze(2).broadcast_to((CD, 8, CD))
                nc.vector.tensor_tensor(ahc3, es[:, :].rearrange(
                    "p (h c) -> p h c", h=8), rseb, A.mult)
                # scatter the 24x24 blocks into the blockdiag layout
                for h in range(NUM_HEADS):
                    adst = a0 if h < 4 else a1
                    r0 = CD * (h % 4)
                    nc.sync.dma_start(adst[r0:r0 + CD, CD * h:CD * (h + 1)],
                                      ahc[:, CD * h:CD * (h + 1)])

                # ---------- M^T = A_bd^T @ W_proj^T ----------
                mt0 = cpool.tile([128, C], f32r, tag="mt0")
                mt1 = cpool.tile([64, C], f32r, tag="mt1")
                pmt0 = psA.tile([128, C], f32, tag="qkv")
                nc.tensor.matmul(pmt0[:, :], a0[:, 0:128], wp0[:, :],
                                 start=True, stop=False)
                nc.tensor.matmul(pmt0[:, :], a1[:, 0:128], wp1[:, :],
                                 start=False, stop=True)
                nc.scalar.copy(mt0[:, :], pmt0[:, :])
                pmt1 = psA.tile([64, C], f32, tag="qkv")
                nc.tensor.matmul(pmt1[:, :], a0[:, 128:192], wp0[:, :],
                                 start=True, stop=False)
                nc.tensor.matmul(pmt1[:, :], a1[:, 128:192], wp1[:, :],
                                 start=False, stop=True)
                nc.scalar.copy(mt1[:, :], pmt1[:, :])

                # ---------- pass 2: out = M @ v (v0 in SBUF, v1 from HBM) ----
                BK = 1024
                vls = {}

                def _load_vl(b0):
                    t = vlpool.tile([64, BK], f32r, tag="vl1")
                    nc.sync.dma_start(t[:, :], vtmp1[:, b0:b0 + BK])
                    vls[b0] = t

                _load_vl(0)
                _load_vl(BK)
                for b0 in range(0, HW, BK):
                    if b0 + 2 * BK < HW:
                        _load_vl(b0 + 2 * BK)
                    vl1 = vls.pop(b0)
                    oa = opool.tile([128, BK], f32, tag="oa")
                    ob = opool.tile([64, BK], f32, tag="ob")
                    for t0 in range(b0, b0 + BK, 512):
                        u0 = t0 - b0
                        pa = psA.tile([128, 512], f32, tag="qkv")
                        nc.tensor.matmul(pa[:, :], mt0[:, 0:128],
                                         vfull0[:, t0:t0 + 512],
                                         start=True, stop=False)
                        nc.tensor.matmul(pa[:, :], mt1[:, 0:128],
                                         vl1[:, u0:u0 + 512],
                                         start=False, stop=True)
                        nc.scalar.copy(oa[:, u0:u0 + 512], pa[:, :])
                        pb = psB.tile([64, 512], f32, tag="dw")
                        nc.tensor.matmul(pb[:, :], mt0[:, 128:192],
                                         vfull0[:, t0:t0 + 512],
                                         start=True, stop=False)
                        nc.tensor.matmul(pb[:, :], mt1[:, 128:192],
                                         vl1[:, u0:u0 + 512],
                                         start=False, stop=True)
                        # DVE in two half-bank reads (a full 2KB PSUM bank
                        # read on DVE crashes the device; 1KB reads are fine)
                        nc.vector.tensor_copy(ob[:, u0:u0 + 256], pb[:, 0:256])
                        nc.vector.tensor_copy(ob[:, u0 + 256:u0 + 512],
                                              pb[:, 256:512])
                    nc.scalar.dma_start(outd[0:128, b0:b0 + BK], oa[:, :])
                    nc.scalar.dma_start(outd[128:192, b0:b0 + BK], ob[:, :])

    nc.compile()
    return nc


def _host_consts(w_qkv, w_dw, w_proj, temperature):
    import ml_dtypes
    fp8 = ml_dtypes.float8_e4m3
    wqT_plain = np.ascontiguousarray(np.asarray(w_qkv, np.float32).T)   # [192, 576]
    wpT = np.ascontiguousarray(np.asarray(w_proj, np.float32).T)        # [192, 192]
    wd = np.asarray(w_dw, dtype=np.float32).reshape(C3, 3, 3)

    # fp8 DoubleRow qkv weights for qk chunks: [128, j, kt, m]
    wq8 = np.zeros((128, 4, 2, 96), dtype=np.float32)
    # fp8 paired-tap dw weights: [96, j, pair, kt, slot]
    dg8 = np.zeros((96, 4, 5, 2, 128), dtype=np.float32)
    for j in range(4):
        chs, slots = _packed_channels(j)
        chs = np.asarray(chs)
        slots = np.asarray(slots)
        wq8[0:128, j, 0, :] = wqT_plain[0:128][:, chs]
        wq8[0:64, j, 1, :] = wqT_plain[128:192][:, chs]
        for p in range(5):
            for kt in range(2):
                t = 2 * p + kt
                if t > 8:
                    continue
                dy, dx = TAPS9[t]
                dg8[np.arange(96), j, p, kt, slots] = wd[chs, dy + 1, dx + 1]

    # v chunk weights
    wqv = np.ascontiguousarray(wqT_plain[:, 384:576])                   # [192, 192]
    dgv = np.zeros((128, 9 * 192), dtype=np.float32)
    for jj, (cb, nch) in enumerate(VCHUNKS):
        for k, (dy, dx) in enumerate(TAPS9):
            col = 9 * 128 * jj + k * nch
            idx = np.arange(nch)
            dgv[idx, col + idx] = wd[384 + cb + idx, dy + 1, dx + 1]

    gmask = np.tile(np.eye(128, dtype=np.float32), (1, 4))              # [128, 512]
    tmapv = np.ones((128, 4), dtype=np.float32)
    tf = np.asarray(temperature, dtype=np.float32).reshape(-1)
    for p in range(4):
        tmapv[32:56, p] = tf[2 * p]
        tmapv[96:120, p] = tf[2 * p + 1]
    eyev = np.eye(128, dtype=np.float32)
    eyebv = np.eye(128).astype(ml_dtypes.bfloat16)
    return dict(wq8=wq8.reshape(128, -1).astype(fp8),
                dg8=dg8.reshape(96, -1).astype(fp8),
                wqv=wqv, dgv=dgv, wpT=wpT, gmask=gmask, tmap=tmapv,
                eye=eyev, eyeb=eyebv)


def kernel(x, w_qkv, w_dw, w_proj, temperature, _trace=False):
    from concourse.bass_utils import run_bass_kernel_spmd

    if "nc" not in _CACHE:
        _CACHE["nc"] = _build()
    nc = _CACHE["nc"]

    consts = _host_consts(w_qkv, w_dw, w_proj, temperature)
    xr = np.ascontiguousarray(np.asarray(x, dtype=np.float32).reshape(NCORES, C, HW))
    in_maps = []
    for b in range(NCORES):
        m = {"x": xr[b]}
        m.update(consts)
        in_maps.append(m)

    try:
        br = run_bass_kernel_spmd(nc, in_maps, core_ids=list(range(NCORES)),
                                  trace=_trace)
    except ModuleNotFoundError:
        br = run_bass_kernel_spmd(nc, in_maps, core_ids=list(range(NCORES)),
                                  trace=False)
    out = np.stack([r["out"] for r in br.results], axis=0).reshape(NCORES, C, H, W)
    _CACHE["last_results"] = br
    return out
